# revision 35
# baseline (speedup 1.0000x reference)
"""Block sliding-window attention on 8 TRN2 NeuronCores.

Sharding: sequence-parallel. 8 shards = (batch b in {0,1}) x (quarter s in
0..3); each core owns 2048 consecutive tokens of one batch plus a 256-token
K/V halo from the previous quarter (zeros + -inf gate for the first quarter).
No collectives: each core computes its tokens' full output rows.

Engine-cost notes driving the layout: every bf16-stationary matmul emits a
separate Ldweights (~146ns serial PE.SEQ); f32r stationaries self-load.
Walrus forbids mixing 16/32-bit matmul operands, and PSUM matmul outputs are
capped at one bank (512 f32). DMA instructions serialize ~625ns on HWDGE.
PE p-state: gaps reset the clock ramp, so smooth pacing matters.

  P1 (all f32r, zero Ldweights): V = hidden @ Wv (256-col Wv panels) ->
      VS scratch; per head ob: KT/QT = W^T @ hiddenT, then RoPE applied
      in-P1 (rotate-half via SBUF partition-swap DMA, in-place DVE mul/add
      on the [128, tokens] head panel) -> roped f32r staging KTSR/QTSR.
  P2+P3 fused per 256-token chunk, no rope left: per head: S^T = K Q^T
      (all f32r, self-loading), exp on ACT -> f32r probs (pgate bias gates
      chunk 0's no-previous case), triangular mask mul in-place on DVE,
      denominator pre-add on Pool + ones-matmul (f32r), O^T = V^T P^T
      (f32r), normalize via DVE reciprocal+mul -> bf16 O, then
      out[chunk] = sum_h O_h @ Wo_h (bf16, resident Wo) in PSUM -> OUT.
"""
import sys

try:
    import concourse  # noqa: F401
except ImportError:
    sys.path.insert(0, '/opt/trn_rl_repo')

import ml_dtypes
import numpy as np

import concourse.bacc as bacc
import concourse.mybir as mybir
import concourse.tile as tile
from concourse.bass_utils import run_bass_kernel_spmd

f32 = mybir.dt.float32
f32r = mybir.dt.float32r
bf16 = mybir.dt.bfloat16
AF = mybir.ActivationFunctionType

DIMS = 2048
HEADS = 16
HD = 128           # head dim
WIN = 256          # window / chunk
B, S = 2, 8192
NSH = 4            # seq shards per batch
THETA = 10000.0
ISQ = float(1.0 / np.sqrt(HD))
IB = DIMS // 128   # 16 input-dim blocks


def tok_tiles(n):
    out, a = [], 0
    while a < n:
        w = min(512, n - a)
        out.append((a, w))
        a += w
    return out


def build(nc, T):
    """Emit the per-core program. T = local tokens (multiple of 512)."""
    TH = T + WIN                      # with halo
    NC_ = T // WIN                    # chunks
    NTB = TH // 128                   # 128-token blocks incl halo
    HT = nc.dram_tensor("HT", [DIMS, TH], f32r, kind="ExternalInput")
    WQ = nc.dram_tensor("WQ", [DIMS, DIMS], f32r, kind="ExternalInput")
    WK = nc.dram_tensor("WK", [DIMS, DIMS], f32r, kind="ExternalInput")
    WV = nc.dram_tensor("WV", [DIMS, DIMS], f32r, kind="ExternalInput")
    WO = nc.dram_tensor("WO", [DIMS, DIMS], bf16, kind="ExternalInput")
    COST = nc.dram_tensor("COST", [HD, TH], bf16, kind="ExternalInput")
    SINT = nc.dram_tensor("SINT", [HD, TH], bf16, kind="ExternalInput")
    TRI23 = nc.dram_tensor("TRI23", [128, 2 * WIN], bf16, kind="ExternalInput")
    PGATE = nc.dram_tensor("PGATE", [128, 1], f32, kind="ExternalInput")
    ONESM = nc.dram_tensor("ONESM", [128, 128], f32r, kind="ExternalInput")
    OUT = nc.dram_tensor("OUT", [T, DIMS], f32, kind="ExternalOutput")

    QTSR = nc.dram_tensor("QTSR", [HEADS, HD, T], f32r)   # roped Q^T
    KTSR = nc.dram_tensor("KTSR", [HEADS, HD, TH], f32r)  # roped K^T (halo)
    VS = nc.dram_tensor("VS", [TH, DIMS], f32r)           # V natural

    with tile.TileContext(nc) as tc:
        with tc.tile_pool(name="cst", bufs=1) as cst:
            tri23 = cst.tile([128, 2 * WIN], bf16)
            pgate = cst.tile([128, 1], f32)
            onesm = cst.tile([128, 128], f32r)
            nc.sync.dma_start(tri23[:], TRI23[:])
            nc.sync.dma_start(pgate[:], PGATE[:])
            nc.sync.dma_start(onesm[:], ONESM[:])

            # ---------------- P1: projections (all f32r) ----------------
            with tc.tile_pool(name="p1", bufs=1) as p1:
                ht = p1.tile([128, IB, TH], f32r)
                htr = HT.rearrange("(ib p) t -> p ib t", p=128)
                NG = 16
                tg = TH // NG
                nc.sync.dma_start(ht[:, :, 0:tg], htr[:, :, 0:tg])

                # V natural: lhsT = hT block [128in, 128tok], rhs = Wv panel
                with tc.tile_pool(name="wv", bufs=2) as wvp, \
                     tc.tile_pool(name="vb", bufs=1) as vbp, \
                     tc.tile_pool(name="vps", bufs=8, space="PSUM") as vps:
                    wvr = WV.rearrange("(ib p) o -> p ib o", p=128)
                    vsr = VS.rearrange("(tb p) c -> p tb c", p=128)
                    for og in range(8):
                        wv = wvp.tile([128, IB, 256], f32r, name="wv")
                        nc.sync.dma_start(
                            wv[:], wvr[:, :, og * 256:(og + 1) * 256])
                        if og == 0:
                            for g in range(1, NG):
                                nc.sync.dma_start(
                                    ht[:, :, g * tg:(g + 1) * tg],
                                    htr[:, :, g * tg:(g + 1) * tg])
                        vbog = vbp.tile([128, NTB, 256], f32r, name="vb")
                        for tb in range(NTB):
                            ps = vps.tile([128, 256], f32, name="vps")
                            for ib in range(IB):
                                nc.tensor.matmul(
                                    ps[:], ht[:, ib, tb * 128:(tb + 1) * 128],
                                    wv[:, ib, :],
                                    start=(ib == 0), stop=(ib == IB - 1))
                            nc.vector.tensor_copy(vbog[:, tb, :], ps[:])
                        nc.scalar.dma_start(
                            vsr[:, :, og * 256:(og + 1) * 256], vbog[:])

                # KT then QT with in-P1 RoPE: lhsT = W block, rhs = hT
                with tc.tile_pool(name="wkq", bufs=2) as wkqp, \
                     tc.tile_pool(name="kst", bufs=2) as kstp, \
                     tc.tile_pool(name="rot", bufs=1) as rotp, \
                     tc.tile_pool(name="cs", bufs=1) as csp, \
                     tc.tile_pool(name="kqps", bufs=8, space="PSUM") as kqps:
                    cosT = csp.tile([128, TH], bf16)
                    sinT = csp.tile([128, TH], bf16)
                    nc.sync.dma_start(cosT[:], COST[:])
                    nc.sync.dma_start(sinT[:], SINT[:])
                    for W_, DST, t0, tlen in ((WK, KTSR, 0, TH),
                                              (WQ, QTSR, WIN, T)):
                        wr = W_.rearrange("(ib p) o -> p ib o", p=128)
                        tts = tok_tiles(tlen)
                        for ob in range(HEADS):
                            wt = wkqp.tile([128, IB, 128], f32r, name="wkq")
                            nc.sync.dma_start(
                                wt[:], wr[:, :, ob * 128:(ob + 1) * 128])
                            st = kstp.tile([128, TH], f32r, name="kst")
                            psums = [kqps.tile([128, 512], f32, name="kqps")
                                     for _ in tts]
                            for ib in range(IB):
                                for ti, (a, w) in enumerate(tts):
                                    nc.tensor.matmul(
                                        psums[ti][:, :w], wt[:, ib, :],
                                        ht[:, ib, t0 + a:t0 + a + w],
                                        start=(ib == 0), stop=(ib == IB - 1))
                            for ti, (a, w) in enumerate(tts):
                                nc.scalar.copy(st[:, a:a + w], psums[ti][:, :w])
                            # rotate-half via SBUF partition-swap, then RoPE
                            # in place on the [128, tlen] head panel
                            rot = rotp.tile([128, TH], f32r, name="rot")
                            nc.sync.dma_start(rot[0:64, :tlen],
                                              st[64:128, :tlen])
                            nc.sync.dma_start(rot[64:128, :tlen],
                                              st[0:64, :tlen])
                            nc.vector.tensor_mul(rot[:, :tlen], rot[:, :tlen],
                                                 sinT[:, t0:t0 + tlen])
                            nc.vector.tensor_mul(st[:, :tlen], st[:, :tlen],
                                                 cosT[:, t0:t0 + tlen])
                            nc.vector.tensor_add(st[:, :tlen], st[:, :tlen],
                                                 rot[:, :tlen])
                            nc.scalar.dma_start(DST[ob][:, :], st[:, :tlen])

            # ---------------- P2 + P3 fused ----------------
            with tc.tile_pool(name="wop", bufs=1) as wop, \
                 tc.tile_pool(name="qk", bufs=2) as qk, \
                 tc.tile_pool(name="ptp", bufs=2) as ptp, \
                 tc.tile_pool(name="pa", bufs=2) as pap, \
                 tc.tile_pool(name="ob", bufs=2) as obp, \
                 tc.tile_pool(name="ot", bufs=9) as otp, \
                 tc.tile_pool(name="so3", bufs=2) as so3p, \
                 tc.tile_pool(name="ps_s", bufs=2, space="PSUM") as ps_s, \
                 tc.tile_pool(name="ps_d", bufs=2, space="PSUM") as ps_d, \
                 tc.tile_pool(name="ps_o", bufs=2, space="PSUM") as ps_o, \
                 tc.tile_pool(name="pp3", bufs=2, space="PSUM") as pp3:
                wo = wop.tile([128, IB, DIMS], bf16)
                wor = WO.rearrange("(ib p) o -> p ib o", p=128)

                def kq_load(SRC, c0, dst):
                    for g in range(4):
                        sg = SRC[g * 4:(g + 1) * 4, :, c0:c0 + WIN]
                        nc.sync.dma_start(dst[:, g * 4:(g + 1) * 4, :],
                                          sg.rearrange("h d w -> d h w"))

                def v_load(c0, vt):
                    src = VS[c0:c0 + WIN].rearrange("(tb p) c -> p tb c",
                                                    p=128)
                    for g in range(4):
                        cs = slice(g * 512, (g + 1) * 512)
                        nc.sync.dma_start(vt[:, :, cs], src[:, :, cs])

                kt_prev = qk.tile([128, HEADS, WIN], f32r, name="kt")
                kq_load(KTSR, 0, kt_prev)
                v_prev = qk.tile([128, 2, DIMS], f32r, name="v")
                v_load(0, v_prev)

                W2 = 2 * WIN
                for c in range(NC_):
                    qt = qk.tile([128, HEADS, WIN], f32r, name="qt")
                    kq_load(QTSR, c * WIN, qt)
                    kt_cur = qk.tile([128, HEADS, WIN], f32r, name="kt")
                    kq_load(KTSR, WIN + c * WIN, kt_cur)
                    v_cur = qk.tile([128, 2, DIMS], f32r, name="v")
                    v_load(WIN + c * WIN, v_cur)
                    if c == 0:
                        # wo quarters, emitted after chunk-0 prep so its bulk
                        # doesn't delay the seam-critical loads
                        for g in range(4):
                            nc.sync.dma_start(
                                wo[:, :, g * 512:(g + 1) * 512],
                                wor[:, :, g * 512:(g + 1) * 512])

                    kts = [kt_prev, kt_prev, kt_cur, kt_cur]
                    vs = [v_prev, v_prev, v_cur, v_cur]
                    ots = []
                    for h0 in range(0, HEADS, 2):
                        pd = ps_d.tile([128, W2], f32, name="pd")
                        po = ps_o.tile([128, W2], f32, name="po")
                        pbs2, pads = [], []
                        for h in (h0, h0 + 1):
                            pbs = []
                            for pr in range(2):
                                ps = ps_s.tile([128, W2], f32, name="ps")
                                for kb2 in range(2):
                                    kb = pr * 2 + kb2
                                    nc.tensor.matmul(
                                        ps[:, kb2 * WIN:(kb2 + 1) * WIN],
                                        kts[kb][:, h,
                                                (kb % 2) * 128:(kb % 2) * 128 + 128],
                                        qt[:, h], start=True, stop=True)
                                pb = ptp.tile([128, W2], f32r, name=f"pt{pr}")
                                if pr == 0:
                                    if c == 0:
                                        nc.scalar.activation(
                                            pb[:], ps[:], AF.Exp,
                                            bias=pgate[:], scale=ISQ)
                                    else:
                                        nc.scalar.activation(
                                            pb[:], ps[:], AF.Exp, scale=ISQ)
                                else:
                                    nc.scalar.activation(pb[:], ps[:], AF.Exp,
                                                         scale=ISQ)
                                    nc.vector.tensor_mul(pb[:], pb[:],
                                                         tri23[:])
                                pbs.append(pb)
                            pbs2.append(pbs)
                            pad = pap.tile([128, W2], f32r, name="pad")
                            nc.gpsimd.tensor_add(pad[:], pbs[0][:], pbs[1][:])
                            pads.append(pad)

                        for i, h in enumerate((h0, h0 + 1)):
                            sl = slice(i * WIN, (i + 1) * WIN)
                            for half in range(2):
                                nc.tensor.matmul(
                                    pd[:, sl], onesm[:],
                                    pads[i][:, half * WIN:(half + 1) * WIN],
                                    start=(half == 0), stop=(half == 1))
                            for kb in range(4):
                                pb = pbs2[i][kb // 2][
                                    :, (kb % 2) * WIN:(kb % 2 + 1) * WIN]
                                nc.tensor.matmul(
                                    po[:, sl],
                                    vs[kb][:, kb % 2, h * 128:(h + 1) * 128],
                                    pb, start=(kb == 0), stop=(kb == 3))
                        rb = obp.tile([128, W2], f32, name="rb")
                        with nc.allow_low_precision("softmax denominator"):
                            nc.vector.reciprocal(rb[:], pd[:])
                        ot = otp.tile([128, W2], bf16, name="ot")
                        nc.vector.tensor_mul(ot[:], po[:], rb[:])
                        ots.append(ot)

                    # P3 for this chunk's 256 tokens
                    for tt in range(2):
                        for nt in range(4):
                            ps3 = pp3.tile([128, 512], f32, name="pp3")
                            for h in range(HEADS):
                                hp, i = divmod(h, 2)
                                lhs = ots[hp][:, i * WIN + tt * 128:
                                              i * WIN + tt * 128 + 128]
                                nc.tensor.matmul(
                                    ps3[:], lhs,
                                    wo[:, h, nt * 512:(nt + 1) * 512],
                                    start=(h == 0), stop=(h == HEADS - 1))
                            so = so3p.tile([128, 512], f32, name="so")
                            nc.scalar.copy(so[:], ps3[:])
                            nc.scalar.dma_start(
                                OUT[c * WIN + tt * 128:
                                    c * WIN + (tt + 1) * 128,
                                    nt * 512:(nt + 1) * 512], so[:])
                    kt_prev, v_prev = kt_cur, v_cur
    return nc


def _host_inputs(hidden_states, Wq, Wk, Wv, Wo, T):
    """Build the 8 per-core input maps."""
    TH = T + WIN
    inv_freq = 1.0 / (THETA ** (np.arange(0, HD, 2, dtype=np.float32) / HD))

    qq = np.arange(WIN)[None, :]
    kk = np.arange(128)[:, None]
    tri23 = np.concatenate([(qq >= kk), (qq >= kk + 128)], 1).astype(
        ml_dtypes.bfloat16)
    onesm_f32 = np.ones((128, 128), np.float32)

    wq32, wk32, wv32 = (np.ascontiguousarray(w, np.float32)
                        for w in (Wq, Wk, Wv))
    wo_bf = np.ascontiguousarray(np.asarray(Wo).astype(ml_dtypes.bfloat16))
    in_maps = []
    for core in range(8):
        b, sh = divmod(core, NSH)
        t0 = sh * T
        hs = np.zeros((TH, DIMS), np.float32)
        lo = max(0, t0 - WIN)
        hs[WIN - (t0 - lo):] = hidden_states[b, lo:t0 + T]
        hT = np.ascontiguousarray(hs.T)

        pos = np.arange(t0 - WIN, t0 + T, dtype=np.float32)
        f = np.outer(inv_freq, pos)                      # [64, TH]
        cos = np.concatenate([np.cos(f), np.cos(f)], 0)  # [128, TH]
        sin = np.sin(f)
        sins = np.concatenate([-sin, sin], 0)
        pg = np.full((128, 1), -1e30 if sh == 0 else 0.0, np.float32)
        in_maps.append({
            "HT": hT, "WQ": wq32, "WK": wk32, "WV": wv32, "WO": wo_bf,
            "COST": np.ascontiguousarray(cos.astype(ml_dtypes.bfloat16)),
            "SINT": np.ascontiguousarray(sins.astype(ml_dtypes.bfloat16)),
            "TRI23": tri23, "PGATE": pg, "ONESM": onesm_f32,
        })
    return in_maps


_CACHE = {}


def run(hidden_states, Wq, Wk, Wv, Wo, T=S // NSH, **spmd_kwargs):
    key = T
    if key not in _CACHE:
        nc = bacc.Bacc(None)
        build(nc, T)
        nc.finalize()
        _CACHE[key] = nc
    nc = _CACHE[key]
    in_maps = _host_inputs(hidden_states, Wq, Wk, Wv, Wo, T)
    res = run_bass_kernel_spmd(nc, in_maps, core_ids=list(range(8)),
                               **spmd_kwargs)
    outs = [res.results[i]["OUT"] for i in range(8)]
    full = np.empty((B, NSH * T, DIMS), np.float32)
    for core in range(8):
        b, sh = divmod(core, NSH)
        full[b, sh * T:(sh + 1) * T] = outs[core]
    return full, res


def kernel(hidden_states, Wq, Wk, Wv, Wo):
    out, _ = run(np.asarray(hidden_states), Wq, Wk, Wv, Wo)
    return out


# revision 41
# speedup vs baseline: 1.0697x; 1.0697x over previous
"""Block sliding-window attention on 8 TRN2 NeuronCores.

Sharding: sequence-parallel. 8 shards = (batch b in {0,1}) x (quarter s in
0..3); each core owns 2048 consecutive tokens of one batch plus a 256-token
K/V halo from the previous quarter (zeros + -inf gate for the first quarter).
No collectives: each core computes its tokens' full output rows.

Engine-cost notes driving the layout: every bf16-stationary matmul emits a
separate Ldweights (~146ns serial PE.SEQ); f32r stationaries self-load.
Walrus forbids mixing 16/32-bit matmul operands, and PSUM matmul outputs are
capped at one bank (512 f32). DMA instructions serialize ~625ns on HWDGE.
PE p-state: gaps reset the clock ramp, so smooth pacing matters.

  P1 (all f32r, zero Ldweights): V = hidden @ Wv (256-col Wv panels) ->
      VS scratch; per head ob: KT/QT = W^T @ hiddenT, then RoPE applied
      in-P1 (rotate-half via SBUF partition-swap DMA, in-place DVE mul/add
      on the [128, tokens] head panel) -> roped f32r staging KTSR/QTSR.
  P2+P3 fused per 256-token chunk, no rope left: per head: S^T = K Q^T
      (all f32r, self-loading), exp on ACT -> f32r probs (pgate bias gates
      chunk 0's no-previous case), triangular mask mul in-place on DVE,
      denominator pre-add on Pool + ones-matmul (f32r), O^T = V^T P^T
      (f32r), normalize via DVE reciprocal+mul -> bf16 O, then
      out[chunk] = sum_h O_h @ Wo_h (bf16, resident Wo) in PSUM -> OUT.
"""
import sys

try:
    import concourse  # noqa: F401
except ImportError:
    sys.path.insert(0, '/opt/trn_rl_repo')

import ml_dtypes
import numpy as np

import concourse.bacc as bacc
import concourse.mybir as mybir
import concourse.tile as tile
from concourse.bass_utils import run_bass_kernel_spmd

f32 = mybir.dt.float32
f32r = mybir.dt.float32r
bf16 = mybir.dt.bfloat16
AF = mybir.ActivationFunctionType

DIMS = 2048
HEADS = 16
HD = 128           # head dim
WIN = 256          # window / chunk
B, S = 2, 8192
NSH = 4            # seq shards per batch
THETA = 10000.0
ISQ = float(1.0 / np.sqrt(HD))
IB = DIMS // 128   # 16 input-dim blocks


def tok_tiles(n):
    out, a = [], 0
    while a < n:
        w = min(512, n - a)
        out.append((a, w))
        a += w
    return out


def build(nc, T):
    """Emit the per-core program. T = local tokens (multiple of 512)."""
    TH = T + WIN                      # with halo
    NC_ = T // WIN                    # chunks
    NTB = TH // 128                   # 128-token blocks incl halo
    HT = nc.dram_tensor("HT", [DIMS, TH], f32r, kind="ExternalInput")
    WQ = nc.dram_tensor("WQ", [DIMS, DIMS], f32r, kind="ExternalInput")
    WK = nc.dram_tensor("WK", [DIMS, DIMS], f32r, kind="ExternalInput")
    WV = nc.dram_tensor("WV", [DIMS, DIMS], f32r, kind="ExternalInput")
    WO = nc.dram_tensor("WO", [DIMS, DIMS], bf16, kind="ExternalInput")
    COST = nc.dram_tensor("COST", [HD, TH], bf16, kind="ExternalInput")
    SINT = nc.dram_tensor("SINT", [HD, TH], bf16, kind="ExternalInput")
    TRI23 = nc.dram_tensor("TRI23", [128, 2 * WIN], bf16, kind="ExternalInput")
    PGATE = nc.dram_tensor("PGATE", [128, 1], f32, kind="ExternalInput")
    ONESM = nc.dram_tensor("ONESM", [128, 128], f32r, kind="ExternalInput")
    OUT = nc.dram_tensor("OUT", [T, DIMS], f32, kind="ExternalOutput")

    QTSR = nc.dram_tensor("QTSR", [HEADS, HD, T], f32r)   # roped Q^T
    KTSR = nc.dram_tensor("KTSR", [HEADS, HD, TH], f32r)  # roped K^T (halo)
    VS = nc.dram_tensor("VS", [TH, DIMS], f32r)           # V natural

    with tile.TileContext(nc) as tc:
        with tc.tile_pool(name="cst", bufs=1) as cst:
            tri23 = cst.tile([128, 2 * WIN], bf16)
            pgate = cst.tile([128, 1], f32)
            onesm = cst.tile([128, 128], f32r)
            nc.sync.dma_start(tri23[:], TRI23[:])
            nc.sync.dma_start(pgate[:], PGATE[:])
            nc.sync.dma_start(onesm[:], ONESM[:])

            # ---------------- P1: projections (all f32r) ----------------
            with tc.tile_pool(name="p1", bufs=1) as p1:
                ht = p1.tile([128, IB, TH], f32r)
                htr = HT.rearrange("(ib p) t -> p ib t", p=128)
                NG = 16
                tg = TH // NG
                nc.sync.dma_start(ht[:, :, 0:tg], htr[:, :, 0:tg])

                # V natural: lhsT = hT block [128in, 128tok], rhs = Wv panel
                with tc.tile_pool(name="wv", bufs=2) as wvp, \
                     tc.tile_pool(name="vb", bufs=1) as vbp, \
                     tc.tile_pool(name="vps", bufs=8, space="PSUM") as vps:
                    wvr = WV.rearrange("(ib p) o -> p ib o", p=128)
                    vsr = VS.rearrange("(tb p) c -> p tb c", p=128)
                    for og in range(8):
                        wv = wvp.tile([128, IB, 256], f32r, name="wv")
                        nc.sync.dma_start(
                            wv[:], wvr[:, :, og * 256:(og + 1) * 256])
                        if og == 0:
                            for g in range(1, NG):
                                nc.sync.dma_start(
                                    ht[:, :, g * tg:(g + 1) * tg],
                                    htr[:, :, g * tg:(g + 1) * tg])
                        vbog = vbp.tile([128, NTB, 256], f32r, name="vb")
                        for tb in range(NTB):
                            ps = vps.tile([128, 256], f32, name="vps")
                            for ib in range(IB):
                                nc.tensor.matmul(
                                    ps[:], ht[:, ib, tb * 128:(tb + 1) * 128],
                                    wv[:, ib, :],
                                    start=(ib == 0), stop=(ib == IB - 1))
                            nc.vector.tensor_copy(vbog[:, tb, :], ps[:])
                        nc.scalar.dma_start(
                            vsr[:, :, og * 256:(og + 1) * 256], vbog[:])

                # KT then QT with in-P1 RoPE: lhsT = W block, rhs = hT
                with tc.tile_pool(name="wkq", bufs=2) as wkqp, \
                     tc.tile_pool(name="kst", bufs=2) as kstp, \
                     tc.tile_pool(name="rot", bufs=1) as rotp, \
                     tc.tile_pool(name="cs", bufs=1) as csp, \
                     tc.tile_pool(name="kqps", bufs=8, space="PSUM") as kqps:
                    cosT = csp.tile([128, TH], bf16)
                    sinT = csp.tile([128, TH], bf16)
                    nc.sync.dma_start(cosT[:], COST[:])
                    nc.sync.dma_start(sinT[:], SINT[:])
                    for W_, DST, t0, tlen in ((WK, KTSR, 0, TH),
                                              (WQ, QTSR, WIN, T)):
                        wr = W_.rearrange("(ib p) o -> p ib o", p=128)
                        tts = tok_tiles(tlen)
                        for ob in range(HEADS):
                            wt = wkqp.tile([128, IB, 128], f32r, name="wkq")
                            nc.sync.dma_start(
                                wt[:], wr[:, :, ob * 128:(ob + 1) * 128])
                            st = kstp.tile([128, TH], f32r, name="kst")
                            psums = [kqps.tile([128, 512], f32, name="kqps")
                                     for _ in tts]
                            for ib in range(IB):
                                for ti, (a, w) in enumerate(tts):
                                    nc.tensor.matmul(
                                        psums[ti][:, :w], wt[:, ib, :],
                                        ht[:, ib, t0 + a:t0 + a + w],
                                        start=(ib == 0), stop=(ib == IB - 1))
                            # rotate-half via SBUF partition-swap, then RoPE
                            # in place; split in token halves so the tail
                            # after the last psum copy stays short
                            rot = rotp.tile([128, TH], f32r, name="rot")
                            nh = (len(tts) + 1) // 2
                            for hs in (tts[:nh], tts[nh:]):
                                if not hs:
                                    continue
                                a0 = hs[0][0]
                                aw = hs[-1][0] + hs[-1][1] - a0
                                for (a, w) in hs:
                                    nc.scalar.copy(
                                        st[:, a:a + w],
                                        psums[tts.index((a, w))][:, :w])
                                sl = slice(a0, a0 + aw)
                                nc.sync.dma_start(rot[0:64, sl],
                                                  st[64:128, sl])
                                nc.sync.dma_start(rot[64:128, sl],
                                                  st[0:64, sl])
                                nc.vector.tensor_mul(
                                    rot[:, sl], rot[:, sl],
                                    sinT[:, t0 + a0:t0 + a0 + aw])
                                nc.vector.tensor_mul(
                                    st[:, sl], st[:, sl],
                                    cosT[:, t0 + a0:t0 + a0 + aw])
                                nc.vector.tensor_add(st[:, sl], st[:, sl],
                                                     rot[:, sl])
                                nc.scalar.dma_start(DST[ob][:, a0:a0 + aw],
                                                    st[:, sl])

            # ---------------- P2 + P3 fused ----------------
            with tc.tile_pool(name="wop", bufs=1) as wop, \
                 tc.tile_pool(name="qk", bufs=2) as qk, \
                 tc.tile_pool(name="ptp", bufs=3) as ptp, \
                 tc.tile_pool(name="pa", bufs=4) as pap, \
                 tc.tile_pool(name="ob", bufs=2) as obp, \
                 tc.tile_pool(name="ot", bufs=9) as otp, \
                 tc.tile_pool(name="so3", bufs=2) as so3p, \
                 tc.tile_pool(name="ps_s", bufs=4, space="PSUM") as ps_s, \
                 tc.tile_pool(name="ps_d", bufs=1, space="PSUM") as ps_d, \
                 tc.tile_pool(name="ps_o", bufs=2, space="PSUM") as ps_o, \
                 tc.tile_pool(name="pp3", bufs=1, space="PSUM") as pp3:
                wo = wop.tile([128, IB, DIMS], bf16)
                wor = WO.rearrange("(ib p) o -> p ib o", p=128)

                def kq_load_g(SRC, c0, dst, g):
                    sg = SRC[g * 4:(g + 1) * 4, :, c0:c0 + WIN]
                    nc.sync.dma_start(dst[:, g * 4:(g + 1) * 4, :],
                                      sg.rearrange("h d w -> d h w"))

                def v_load_g(c0, vt, g):
                    src = VS[c0:c0 + WIN].rearrange("(tb p) c -> p tb c",
                                                    p=128)
                    cs = slice(g * 512, (g + 1) * 512)
                    nc.sync.dma_start(vt[:, :, cs], src[:, :, cs])

                kt_prev = qk.tile([128, HEADS, WIN], f32r, name="kt")
                v_prev = qk.tile([128, 2, DIMS], f32r, name="v")

                W2 = 2 * WIN
                for c in range(NC_):
                    qt = qk.tile([128, HEADS, WIN], f32r, name="qt")
                    kt_cur = qk.tile([128, HEADS, WIN], f32r, name="kt")
                    v_cur = qk.tile([128, 2, DIMS], f32r, name="v")
                    # group-round-robin loads: everything head-group g needs
                    # lands before any of group g+1, so compute on the first
                    # heads starts after a fraction of the transfer bytes
                    for g in range(4):
                        kq_load_g(QTSR, c * WIN, qt, g)
                        if c == 0:
                            kq_load_g(KTSR, 0, kt_prev, g)
                            v_load_g(0, v_prev, g)
                        kq_load_g(KTSR, WIN + c * WIN, kt_cur, g)
                        v_load_g(WIN + c * WIN, v_cur, g)
                        if c == 0 and g >= 2:
                            for gg in (2 * g - 4, 2 * g - 3):
                                nc.sync.dma_start(
                                    wo[:, :, gg * 512:(gg + 1) * 512],
                                    wor[:, :, gg * 512:(gg + 1) * 512])

                    kts = [kt_prev, kt_prev, kt_cur, kt_cur]
                    vs = [v_prev, v_prev, v_cur, v_cur]
                    ots = []

                    def pair_scores(h0):
                        pbs2, pads = [], []
                        for h in (h0, h0 + 1):
                            pbs = []
                            for pr in range(2):
                                ps = ps_s.tile([128, W2], f32, name="ps")
                                for kb2 in range(2):
                                    kb = pr * 2 + kb2
                                    nc.tensor.matmul(
                                        ps[:, kb2 * WIN:(kb2 + 1) * WIN],
                                        kts[kb][:, h,
                                                (kb % 2) * 128:(kb % 2) * 128 + 128],
                                        qt[:, h], start=True, stop=True)
                                pb = ptp.tile([128, W2], f32r, name=f"pt{pr}")
                                if pr == 0 and c == 0:
                                    nc.scalar.activation(
                                        pb[:], ps[:], AF.Exp,
                                        bias=pgate[:], scale=ISQ)
                                else:
                                    nc.scalar.activation(pb[:], ps[:], AF.Exp,
                                                         scale=ISQ)
                                    if pr == 1:
                                        nc.vector.tensor_mul(pb[:], pb[:],
                                                             tri23[:])
                                pbs.append(pb)
                            pbs2.append(pbs)
                            pad = pap.tile([128, W2], f32r, name="pad")
                            nc.gpsimd.tensor_add(pad[:], pbs[0][:], pbs[1][:])
                            pads.append(pad)
                        return pbs2, pads

                    def pair_out(h0, pbs2, pads):
                        pd = ps_d.tile([128, W2], f32, name="pd")
                        po = ps_o.tile([128, W2], f32, name="po")
                        for i, h in enumerate((h0, h0 + 1)):
                            sl = slice(i * WIN, (i + 1) * WIN)
                            for half in range(2):
                                nc.tensor.matmul(
                                    pd[:, sl], onesm[:],
                                    pads[i][:, half * WIN:(half + 1) * WIN],
                                    start=(half == 0), stop=(half == 1))
                            for kb in range(4):
                                pb = pbs2[i][kb // 2][
                                    :, (kb % 2) * WIN:(kb % 2 + 1) * WIN]
                                nc.tensor.matmul(
                                    po[:, sl],
                                    vs[kb][:, kb % 2, h * 128:(h + 1) * 128],
                                    pb, start=(kb == 0), stop=(kb == 3))
                        rb = obp.tile([128, W2], f32, name="rb")
                        with nc.allow_low_precision("softmax denominator"):
                            nc.vector.reciprocal(rb[:], pd[:])
                        ot = otp.tile([128, W2], bf16, name="ot")
                        nc.vector.tensor_mul(ot[:], po[:], rb[:])
                        ots.append(ot)

                    # one-pair lookahead: emit pair j+1's scores before pair
                    # j's denominator/PV so the PE never waits on the
                    # exp -> mask -> pool-add chain
                    pend = None
                    for h0 in range(0, HEADS, 2):
                        cur = (h0,) + pair_scores(h0)
                        if pend is not None:
                            pair_out(*pend)
                        pend = cur
                    pair_out(*pend)

                    # P3 for this chunk's 256 tokens
                    for tt in range(2):
                        for nt in range(4):
                            ps3 = pp3.tile([128, 512], f32, name="pp3")
                            for h in range(HEADS):
                                hp, i = divmod(h, 2)
                                lhs = ots[hp][:, i * WIN + tt * 128:
                                              i * WIN + tt * 128 + 128]
                                nc.tensor.matmul(
                                    ps3[:], lhs,
                                    wo[:, h, nt * 512:(nt + 1) * 512],
                                    start=(h == 0), stop=(h == HEADS - 1))
                            so = so3p.tile([128, 512], f32, name="so")
                            nc.scalar.copy(so[:], ps3[:])
                            nc.scalar.dma_start(
                                OUT[c * WIN + tt * 128:
                                    c * WIN + (tt + 1) * 128,
                                    nt * 512:(nt + 1) * 512], so[:])
                    kt_prev, v_prev = kt_cur, v_cur
    return nc


def _host_inputs(hidden_states, Wq, Wk, Wv, Wo, T):
    """Build the 8 per-core input maps."""
    TH = T + WIN
    inv_freq = 1.0 / (THETA ** (np.arange(0, HD, 2, dtype=np.float32) / HD))

    qq = np.arange(WIN)[None, :]
    kk = np.arange(128)[:, None]
    tri23 = np.concatenate([(qq >= kk), (qq >= kk + 128)], 1).astype(
        ml_dtypes.bfloat16)
    onesm_f32 = np.ones((128, 128), np.float32)

    wq32, wk32, wv32 = (np.ascontiguousarray(w, np.float32)
                        for w in (Wq, Wk, Wv))
    wo_bf = np.ascontiguousarray(np.asarray(Wo).astype(ml_dtypes.bfloat16))
    in_maps = []
    for core in range(8):
        b, sh = divmod(core, NSH)
        t0 = sh * T
        hs = np.zeros((TH, DIMS), np.float32)
        lo = max(0, t0 - WIN)
        hs[WIN - (t0 - lo):] = hidden_states[b, lo:t0 + T]
        hT = np.ascontiguousarray(hs.T)

        pos = np.arange(t0 - WIN, t0 + T, dtype=np.float32)
        f = np.outer(inv_freq, pos)                      # [64, TH]
        cos = np.concatenate([np.cos(f), np.cos(f)], 0)  # [128, TH]
        sin = np.sin(f)
        sins = np.concatenate([-sin, sin], 0)
        pg = np.full((128, 1), -1e30 if sh == 0 else 0.0, np.float32)
        in_maps.append({
            "HT": hT, "WQ": wq32, "WK": wk32, "WV": wv32, "WO": wo_bf,
            "COST": np.ascontiguousarray(cos.astype(ml_dtypes.bfloat16)),
            "SINT": np.ascontiguousarray(sins.astype(ml_dtypes.bfloat16)),
            "TRI23": tri23, "PGATE": pg, "ONESM": onesm_f32,
        })
    return in_maps


_CACHE = {}


def run(hidden_states, Wq, Wk, Wv, Wo, T=S // NSH, **spmd_kwargs):
    key = T
    if key not in _CACHE:
        nc = bacc.Bacc(None)
        build(nc, T)
        nc.finalize()
        _CACHE[key] = nc
    nc = _CACHE[key]
    in_maps = _host_inputs(hidden_states, Wq, Wk, Wv, Wo, T)
    res = run_bass_kernel_spmd(nc, in_maps, core_ids=list(range(8)),
                               **spmd_kwargs)
    outs = [res.results[i]["OUT"] for i in range(8)]
    full = np.empty((B, NSH * T, DIMS), np.float32)
    for core in range(8):
        b, sh = divmod(core, NSH)
        full[b, sh * T:(sh + 1) * T] = outs[core]
    return full, res


def kernel(hidden_states, Wq, Wk, Wv, Wo):
    out, _ = run(np.asarray(hidden_states), Wq, Wk, Wv, Wo)
    return out
